# revision 48
# baseline (speedup 1.0000x reference)
"""Causal attention (with faithful missing-head-transpose reshape bug) on 8 Trainium2 cores.

Problem: B=2, T=2048, E=1024, H=16, dk=64.
  qkv = x @ w_qkv.T ; q,k,v split; per-head causal softmax attention;
  out = att_out[B,H,T,dk].reshape(B,T,E)  (NO head transpose — faithful bug);
  y = out @ w_proj.T + b_proj

Because of the missing transpose, output rows y[b, 128h : 128h+128, :] depend
ONLY on head h.  Sharding (batch x head-group) over 8 cores therefore needs NO
collectives: core c handles batch c//4 and heads 4*(c%4) .. 4*(c%4)+3.

This version is a fully software-pipelined single-stream design:
  - all work (QKV projections, scores, exp, P@V, normalization, output
    projection) is emitted into ONE interleaved PE instruction stream so the
    tensor engine never idles (HAM clock gate stays at 2.4 GHz) and no
    zero-matmul warmers are needed;
  - heads are processed in PAIRS (2 heads x 4 query windows each); pair 0's
    output projection runs as filler inside pair 1's attention loop;
  - QKV-projection matmuls are closures in a "filler queue", popped between
    attention steps; window prerequisites are force-drained at window starts;
  - input DMA is issued in chunk-priority order (wk/wq + x cols 0:512 first)
    so the first matmul can start ~1-2us in;
  - causal masking via gpsimd(Pool-engine) affine_select directly on the exp
    tiles (DVE freed for PSUM drains); diagonal blocks with qq in (1,2) are
    column-trimmed in scores/exp/PV (skip fully-masked columns);
  - V carries a ones-column per head so row 64 of each P@V PSUM tile is the
    softmax denominator for free; reciprocal is spread across lanes via a
    DMA round-trip and broadcast back with a K=1 matmul.
"""

import os
import sys
from collections import deque

import numpy as np

for _p in ("/opt/trn_rl_repo", "/root/.axon_site/_ro/trn_rl_repo"):
    if os.path.isdir(_p) and _p not in sys.path:
        sys.path.insert(0, _p)

import ml_dtypes  # noqa: E402

import concourse.bacc as bacc  # noqa: E402
import concourse.mybir as mybir  # noqa: E402
from concourse.bass import ds, ts  # noqa: E402
from concourse.tile import TileContext  # noqa: E402

F32 = mybir.dt.float32
BF16 = mybir.dt.bfloat16
AF = mybir.ActivationFunctionType
BF16NP = ml_dtypes.bfloat16

P = 128
E = 1024
DK = 64
HPC = 4  # heads per core
NPAIR = 2  # head pairs per core
TW = 512  # i-window for scores / pv matmuls
EC = E // P  # 8 e-chunks
FW = E // 512  # 2 output-feature windows


def build_nc(T=2048):
    W = T // TW  # query windows (4)
    JPW = TW // P  # j-chunks per window (4)
    TC = T // P  # t-chunks for V (16)
    RR = (T * DK) // E  # rows of R per head (128)
    TT = E // DK  # 16 t-positions per R row
    NSP = 2 * TW // P  # denom elems per lane after spread (8)

    nc = bacc.Bacc("TRN2", target_bir_lowering=False, debug=False)
    xT = nc.declare_dram_parameter("xT", [E, T], BF16, isOutput=False)
    wqT = nc.declare_dram_parameter("wqT", [E, HPC * DK], BF16, isOutput=False)
    wkT = nc.declare_dram_parameter("wkT", [E, HPC * DK], BF16, isOutput=False)
    wvT = nc.declare_dram_parameter("wvT", [E, HPC * DK], BF16, isOutput=False)
    wpT = nc.declare_dram_parameter("wpT", [E, E], BF16, isOutput=False)
    bp = nc.declare_dram_parameter("bp", [1, E], BF16, isOutput=False)
    y = nc.declare_dram_parameter("y", [HPC * RR, E], BF16, isOutput=True)

    with nc.allow_low_precision(reason="bf16 matmuls; accumulation stays fp32 in PSUM"), TileContext(nc) as tc:
        with (
            tc.tile_pool(name="const", bufs=1) as const,
            tc.tile_pool(name="wts", bufs=1) as wts,
            tc.tile_pool(name="xin", bufs=1) as xin,
            tc.tile_pool(name="qkv", bufs=1) as qkv_pool,
            tc.tile_pool(name="att", bufs=1) as att_pool,
            tc.tile_pool(name="exps", bufs=6) as epool,
            tc.tile_pool(name="rec", bufs=2) as rpool,
            tc.tile_pool(name="yout", bufs=2) as ypool,
            tc.tile_pool(name="psq", bufs=2, space="PSUM") as psq,
            tc.tile_pool(name="psa", bufs=1, space="PSUM") as psa,
        ):
            ones = const.tile([P, P], BF16)
            nc.vector.memset(ones, 1.0)
            zer = const.tile([P, P], BF16)
            nc.vector.memset(zer, 0.0)
            wsrc = const.tile([P, TW], BF16)
            nc.vector.memset(wsrc, 0.0)
            bp_sb = const.tile([1, E], BF16)

            wq_sb = wts.tile([P, EC, HPC * DK], BF16)
            wk_sb = wts.tile([P, EC, HPC * DK], BF16)
            wv_sb = wts.tile([P, EC, HPC * DK], BF16)
            wp_sb = wts.tile([P, EC, E], BF16)
            xp = xin.tile([P, EC, T], BF16)

            qT = qkv_pool.tile([P, NPAIR, T], BF16)
            kT = qkv_pool.tile([P, NPAIR, T], BF16)
            vsb = qkv_pool.tile([P, TC, HPC * (DK + 1)], BF16)
            # ones column per head (row 64 of each P@V psum = softmax denominator)
            nc.vector.memset(
                vsb.rearrange("p t (h c) -> p t h c", c=DK + 1)[:, :, :, DK : DK + 1], 1.0
            )

            att2 = []
            for h in range(HPC):
                a = att_pool.tile([P, T], BF16, name=f"att2_{h}", tag=f"att2_{h}")
                att2.append(a)
                # last col of shifted half never written; keep sim happy
                nc.vector.memset(a[DK : 2 * DK, T - 1 : T], 0.0)

            # ---------------- input DMA (priority-chunked) ----------------
            dq = [nc.sync, nc.gpsimd, nc.scalar]
            _di = [0]

            def dma_in(out, in_):
                dq[_di[0] % 3].dma_start(out=out, in_=in_)
                _di[0] += 1

            # quadruplets (wk_e, wq_e, x_e[0:512], wv_e) so the first kT matmul
            # can start after ~256KB of traffic and V matmuls shortly after
            for e in range(EC):
                dma_in(wk_sb[:, e, :], wkT[ts(e, P), :])
                dma_in(wq_sb[:, e, :], wqT[ts(e, P), :])
                dma_in(xp[:, e, 0:TW], xT[ts(e, P), 0:TW])
                dma_in(wv_sb[:, e, :], wvT[ts(e, P), :])
            dma_in(bp_sb, bp[:, :])
            for e in range(EC):
                dma_in(xp[:, e, TW : 2 * TW], xT[ts(e, P), TW : 2 * TW])
            for e in range(EC):
                dma_in(xp[:, e, 2 * TW : 4 * TW], xT[ts(e, P), 2 * TW : 4 * TW])
            for e in range(EC):
                dma_in(wp_sb[:, e, :], wpT[ts(e, P), :])

            # ---------------- filler queue ----------------
            fillers = deque()
            n_added = [0]
            n_popped = [0]
            need = {}

            def add_f(fn):
                fillers.append(fn)
                n_added[0] += 1

            def take(n):
                while n > 0 and fillers:
                    fillers.popleft()()
                    n_popped[0] += 1
                    n -= 1

            def drain_to(k):
                while n_popped[0] < k and fillers:
                    fillers.popleft()()
                    n_popped[0] += 1

            def add_kq_group(dst, wsb, p, w):
                cell = {}

                for e in range(EC):
                    def mm(e=e, cell=cell, dst=dst, wsb=wsb, p=p, w=w):
                        if e == 0:
                            cell["ps"] = psq.tile([P, TW], F32, tag="qa", name="ps_qk")
                        nc.tensor.matmul(
                            cell["ps"],
                            wsb[:, e, ts(p, P)],
                            xp[:, e, ds(TW * w, TW)],
                            start=(e == 0),
                            stop=(e == EC - 1),
                        )

                    add_f(mm)

                def cp(cell=cell, dst=dst, p=p, w=w):
                    nc.vector.tensor_copy(dst[:, p, ds(TW * w, TW)], cell["ps"])

                add_f(cp)

            def add_v_group(t):
                cell = {}

                for e in range(EC):
                    def mm(e=e, cell=cell, t=t):
                        if e == 0:
                            cell["ps"] = psq.tile([P, HPC * DK], F32, tag="qa", name="ps_v")
                        nc.tensor.matmul(
                            cell["ps"],
                            xp[:, e, ts(t, P)],
                            wv_sb[:, e, :],
                            start=(e == 0),
                            stop=(e == EC - 1),
                        )

                    add_f(mm)

                def cp(cell=cell, t=t):
                    nc.vector.tensor_copy(
                        vsb.rearrange("p t (h c) -> p t h c", c=DK + 1)[:, t, :, 0:DK],
                        cell["ps"].rearrange("p (h d) -> p h d", d=DK),
                    )

                add_f(cp)

            def add_proj(h):
                a2v = att2[h].rearrange("p (r t) -> p r t", t=TT)
                for fw in range(FW):
                    cell = {}
                    for m in range(EC):
                        def mm(m=m, fw=fw, cell=cell, a2v=a2v):
                            if m == 0:
                                cell["yp"] = psq.tile([P, TW], F32, tag="qa", name="yp")
                            nc.tensor.matmul(
                                cell["yp"][0:RR, :],
                                a2v[:, :, 2 * m : 2 * m + 1],
                                wp_sb[:, m, ds(512 * fw, 512)],
                                start=(m == 0),
                                stop=False,
                            )

                        add_f(mm)

                    def mmb(fw=fw, cell=cell):
                        nc.tensor.matmul(
                            cell["yp"][0:RR, :],
                            ones[0:1, 0:RR],
                            bp_sb[0:1, ds(512 * fw, 512)],
                            start=False,
                            stop=True,
                        )

                    add_f(mmb)

                    def cpd(fw=fw, cell=cell, h=h):
                        ysb = ypool.tile([P, 512], BF16, name="ysb")
                        nc.vector.tensor_copy(ysb[0:RR, :], cell["yp"][0:RR, :])
                        nc.sync.dma_start(
                            out=y[ds(RR * h, RR), ds(512 * fw, 512)], in_=ysb[0:RR, :]
                        )

                    add_f(cpd)

            def add_kq(p, w):
                add_kq_group(kT, wk_sb, p, w)
                add_kq_group(qT, wq_sb, p, w)

            add_kq(0, 0)
            for t in range(0, 4):
                add_v_group(t)
            need[(0, 0)] = n_added[0]
            add_kq(0, 1)
            add_kq(1, 0)
            need[(1, 0)] = n_added[0]
            for t in range(4, 8):
                add_v_group(t)
            need[(0, 1)] = n_added[0]
            add_kq(0, 2)
            add_kq(1, 1)
            for t in range(8, 12):
                add_v_group(t)
            need[(0, 2)] = n_added[0]
            need[(1, 1)] = n_added[0]
            add_kq(0, 3)
            add_kq(1, 2)
            for t in range(12, 16):
                add_v_group(t)
            need[(0, 3)] = n_added[0]
            need[(1, 2)] = n_added[0]
            add_kq(1, 3)
            need[(1, 3)] = n_added[0]

            # ---------------- attention (pair-outer, pipelined) ----------------
            pending_norm = [None, None]

            for p in range(NPAIR):
                for w in range(W):
                    drain_to(need[(p, w)])
                    njc = JPW * (w + 1)
                    pvt = [
                        psa.tile([P, TW], F32, tag=f"pv{hl}", bufs=1, name=f"pv{hl}")
                        for hl in range(2)
                    ]
                    ess = {}

                    def emit_scores(jc, p=p, w=w, ess=ess):
                        qq = jc - JPW * w
                        trim = qq in (1, 2)
                        i0 = P * qq if trim else 0
                        st = psa.tile([P, 2 * TW], F32, tag="s", bufs=2, name="st")
                        for sub in range(2):
                            nc.tensor.matmul(
                                st[:, ds(TW * sub + i0, TW - i0)],
                                kT[ds(DK * sub, DK), p, ts(jc, P)],
                                qT[ds(DK * sub, DK), p, ds(TW * w + i0, TW - i0)],
                                start=True,
                                stop=True,
                            )
                        es = epool.tile([P, 2 * TW], BF16, name="es")
                        if trim:
                            for sub in range(2):
                                sl = ds(TW * sub + i0, TW - i0)
                                nc.scalar.activation(es[:, sl], st[:, sl], AF.Exp, scale=1.0 / 8.0)
                        else:
                            nc.scalar.activation(es, st, AF.Exp, scale=1.0 / 8.0)
                        if qq >= 0:
                            for sub in range(2):
                                sl = ds(TW * sub + i0, TW - i0)
                                nc.gpsimd.affine_select(
                                    out=es[:, sl],
                                    in_=es[:, sl],
                                    pattern=[[1, TW - i0]],
                                    compare_op=mybir.AluOpType.is_ge,
                                    fill=0.0,
                                    base=-(P * qq - i0),
                                    channel_multiplier=-1,
                                )
                        ess[jc] = (es, qq)

                    def emit_pv(jc, p=p, w=w, njc=njc, pvt=pvt, ess=ess):
                        es, qq = ess.pop(jc)
                        trim = qq in (1, 2)
                        i0 = P * qq if trim else 0
                        for hl in range(2):
                            h4 = 2 * p + hl
                            nc.tensor.matmul(
                                pvt[hl][0 : DK + 1, ds(i0, TW - i0)],
                                vsb[:, jc, ds((DK + 1) * h4, DK + 1)],
                                es[:, ds(TW * hl + i0, TW - i0)],
                                start=(jc == 0 and not trim),
                                stop=(jc == njc - 1 and not trim),
                                skip_group_check=trim,
                            )

                    for step in range(njc + 2):
                        if step < njc:
                            emit_scores(step)
                        take(2)
                        if step >= 2:
                            emit_pv(step - 2)
                        take(1)
                        if step == njc + 1 and pending_norm[1] is not None:
                            pending_norm[1]()  # rt broadcast + normalize + shift
                            pending_norm[1] = None

                    # ---- window drain: praw + denominators ----
                    dns = rpool.tile([P, 2 * TW], F32, name="dns", tag="dns")
                    praws = []
                    for hl in range(2):
                        praw = rpool.tile([P, TW], BF16, name="praw", tag=f"praw{hl}", bufs=2)
                        nc.vector.tensor_copy(praw[0:DK, :], pvt[hl][0:DK, :])
                        nc.vector.tensor_copy(
                            dns[DK : DK + 1, ds(TW * hl, TW)], pvt[hl][DK : DK + 1, :]
                        )
                        praws.append(praw)

                    recb = rpool.tile([P, 2 * TW], BF16, name="recb", tag="recb", bufs=2)

                    def norm_chain(p=p, w=w, dns=dns, recb=recb):
                        sp = rpool.tile([P, 2 * NSP], F32, name="sp", tag="sp")
                        nc.sync.dma_start(
                            out=sp[:, 0:NSP],
                            in_=dns[DK : DK + 1, :].rearrange("a (p c) -> a p c", c=NSP),
                        )
                        nc.vector.reciprocal(out=sp[:, NSP : 2 * NSP], in_=sp[:, 0:NSP])
                        spb = rpool.tile([P, 2 * NSP], BF16, name="spb", tag="spb")
                        nc.vector.tensor_copy(spb[:, 0:NSP], sp[:, NSP : 2 * NSP])
                        nc.sync.dma_start(
                            out=recb[DK : DK + 1, :].rearrange("a (p c) -> a p c", c=NSP),
                            in_=spb[:, 0:NSP],
                        )

                    def norm_pe(p=p, w=w, recb=recb, praws=praws):
                        for hl in range(2):
                            h = 2 * p + hl
                            rt = psa.tile([P, 2 * TW], F32, tag="s", bufs=2, name="rt")
                            nc.tensor.matmul(
                                rt[0:DK, 0:TW],
                                ones[DK : DK + 1, 0:DK],
                                recb[DK : DK + 1, ds(TW * hl, TW)],
                                start=True,
                                stop=True,
                            )
                            nc.vector.tensor_mul(
                                att2[h][0:DK, ds(TW * w, TW)],
                                rt[0:DK, 0:TW],
                                praws[hl][0:DK, :],
                            )
                            if w == 0:
                                nc.sync.dma_start(
                                    out=att2[h][DK : 2 * DK, 0 : TW - 1],
                                    in_=att2[h][0:DK, 1:TW],
                                )
                            else:
                                nc.sync.dma_start(
                                    out=att2[h][DK : 2 * DK, TW * w - 1 : TW * (w + 1) - 1],
                                    in_=att2[h][0:DK, ds(TW * w, TW)],
                                )
                            if w == W - 1:
                                # head complete: its projection becomes filler
                                add_proj(h)

                    norm_chain()  # reciprocal chain starts right away (DVE/DMA only)
                    pending_norm[1] = norm_pe

            # ---------------- tail ----------------
            # keep PE warm through the last norm chain, then flush projections
            take(10**9)

            def warm(n):
                wtf = psa.tile([P, 2 * TW], F32, tag="s", bufs=2, name="wtf")
                for i in range(n):
                    nc.tensor.matmul(
                        wtf[0 : DK + 1, 0:TW],
                        zer[:, 0 : DK + 1],
                        wsrc,
                        start=(i == 0),
                        stop=(i == n - 1),
                    )

            warm(10)
            pending_norm[1]()  # last window's rt + normalize, appends projections
            pending_norm[1] = None
            warm(6)
            take(10**9)
    nc.compile()
    return nc


_CACHE = {}
LAST_RESULT = None


def _get_nc(T=2048):
    key = ("nc", T)
    if key not in _CACHE:
        _CACHE[key] = build_nc(T=T)
    return _CACHE[key]


def make_in_maps(x, w_qkv, w_proj, b_proj):
    B, T, _E = x.shape
    in_maps = []
    wpTh = np.ascontiguousarray(w_proj.T.astype(BF16NP))
    bph = np.ascontiguousarray(b_proj.reshape(1, E).astype(BF16NP))
    xTs = [np.ascontiguousarray(x[b].T.astype(BF16NP)) for b in range(B)]
    for c in range(8):
        b, g = divmod(c, 4)
        r0 = HPC * DK * g  # 256*g
        sl = slice(r0, r0 + HPC * DK)
        in_maps.append(
            {
                "xT": xTs[b],
                "wqT": np.ascontiguousarray(w_qkv[sl, :].T.astype(BF16NP)),
                "wkT": np.ascontiguousarray(w_qkv[E:][sl, :].T.astype(BF16NP)),
                "wvT": np.ascontiguousarray(w_qkv[2 * E :][sl, :].T.astype(BF16NP)),
                "wpT": wpTh,
                "bp": bph,
            }
        )
    return in_maps


def kernel(x, w_qkv, w_proj, b_proj):
    global LAST_RESULT
    from concourse.bass_utils import run_bass_kernel_spmd

    x = np.asarray(x, dtype=np.float32)
    w_qkv = np.asarray(w_qkv, dtype=np.float32)
    w_proj = np.asarray(w_proj, dtype=np.float32)
    b_proj = np.asarray(b_proj, dtype=np.float32)
    B, T, _E = x.shape

    nc = _get_nc(T=T)
    in_maps = make_in_maps(x, w_qkv, w_proj, b_proj)
    res = run_bass_kernel_spmd(nc, in_maps, core_ids=list(range(8)))
    LAST_RESULT = res

    out = np.empty((B, T, E), dtype=np.float32)
    rows = HPC * ((T * DK) // E)  # 512 rows per core
    for c in range(8):
        b, g = divmod(c, 4)
        out[b, rows * g : rows * (g + 1), :] = np.asarray(res.results[c]["y"]).astype(np.float32)
    return out


# revision 50
# speedup vs baseline: 1.0243x; 1.0243x over previous
"""Causal attention (with faithful missing-head-transpose reshape bug) on 8 Trainium2 cores.

Problem: B=2, T=2048, E=1024, H=16, dk=64.
  qkv = x @ w_qkv.T ; q,k,v split; per-head causal softmax attention;
  out = att_out[B,H,T,dk].reshape(B,T,E)  (NO head transpose — faithful bug);
  y = out @ w_proj.T + b_proj

Because of the missing transpose, output rows y[b, 128h : 128h+128, :] depend
ONLY on head h.  Sharding (batch x head-group) over 8 cores therefore needs NO
collectives: core c handles batch c//4 and heads 4*(c%4) .. 4*(c%4)+3.

This version is a fully software-pipelined single-stream design:
  - all work (QKV projections, scores, exp, P@V, normalization, output
    projection) is emitted into ONE interleaved PE instruction stream so the
    tensor engine never idles (HAM clock gate stays at 2.4 GHz) and no
    zero-matmul warmers are needed;
  - heads are processed in PAIRS (2 heads x 4 query windows each); pair 0's
    output projection runs as filler inside pair 1's attention loop;
  - QKV-projection matmuls are closures in a "filler queue", popped between
    attention steps; window prerequisites are force-drained at window starts;
  - input DMA is issued in chunk-priority order (wk/wq + x cols 0:512 first)
    so the first matmul can start ~1-2us in;
  - causal masking via gpsimd(Pool-engine) affine_select directly on the exp
    tiles (DVE freed for PSUM drains); diagonal blocks with qq in (1,2) are
    column-trimmed in scores/exp/PV (skip fully-masked columns);
  - V carries a ones-column per head so row 64 of each P@V PSUM tile is the
    softmax denominator for free; reciprocal is spread across lanes via a
    DMA round-trip and broadcast back with a K=1 matmul.
"""

import os
import sys
from collections import deque

import numpy as np

for _p in ("/opt/trn_rl_repo", "/root/.axon_site/_ro/trn_rl_repo"):
    if os.path.isdir(_p) and _p not in sys.path:
        sys.path.insert(0, _p)

import ml_dtypes  # noqa: E402

import concourse.bacc as bacc  # noqa: E402
import concourse.mybir as mybir  # noqa: E402
from concourse.bass import ds, ts  # noqa: E402
from concourse.tile import TileContext  # noqa: E402

F32 = mybir.dt.float32
BF16 = mybir.dt.bfloat16
AF = mybir.ActivationFunctionType
BF16NP = ml_dtypes.bfloat16

P = 128
E = 1024
DK = 64
HPC = 4  # heads per core
NPAIR = 2  # head pairs per core
TW = 512  # i-window for scores / pv matmuls
EC = E // P  # 8 e-chunks
FW = E // 512  # 2 output-feature windows


def build_nc(T=2048):
    W = T // TW  # query windows (4)
    JPW = TW // P  # j-chunks per window (4)
    TC = T // P  # t-chunks for V (16)
    RR = (T * DK) // E  # rows of R per head (128)
    TT = E // DK  # 16 t-positions per R row
    NSP = 2 * TW // P  # denom elems per lane after spread (8)

    nc = bacc.Bacc("TRN2", target_bir_lowering=False, debug=False)
    xT = nc.declare_dram_parameter("xT", [E, T], BF16, isOutput=False)
    wqT = nc.declare_dram_parameter("wqT", [E, HPC * DK], BF16, isOutput=False)
    wkT = nc.declare_dram_parameter("wkT", [E, HPC * DK], BF16, isOutput=False)
    wvT = nc.declare_dram_parameter("wvT", [E, HPC * DK], BF16, isOutput=False)
    wpT = nc.declare_dram_parameter("wpT", [E, E], BF16, isOutput=False)
    bp = nc.declare_dram_parameter("bp", [1, E], BF16, isOutput=False)
    y = nc.declare_dram_parameter("y", [HPC * RR, E], BF16, isOutput=True)

    with nc.allow_low_precision(reason="bf16 matmuls; accumulation stays fp32 in PSUM"), TileContext(nc) as tc:
        with (
            tc.tile_pool(name="const", bufs=1) as const,
            tc.tile_pool(name="wts", bufs=1) as wts,
            tc.tile_pool(name="xin", bufs=1) as xin,
            tc.tile_pool(name="qkv", bufs=1) as qkv_pool,
            tc.tile_pool(name="att", bufs=1) as att_pool,
            tc.tile_pool(name="exps", bufs=6) as epool,
            tc.tile_pool(name="rec", bufs=2) as rpool,
            tc.tile_pool(name="yout", bufs=2) as ypool,
            tc.tile_pool(name="psq", bufs=2, space="PSUM") as psq,
            tc.tile_pool(name="psa", bufs=1, space="PSUM") as psa,
        ):
            ones = const.tile([P, P], BF16)
            nc.vector.memset(ones, 1.0)
            zer = const.tile([P, P], BF16)
            nc.vector.memset(zer, 0.0)
            wsrc = const.tile([P, TW], BF16)
            nc.vector.memset(wsrc, 0.0)
            bp_sb = const.tile([1, E], BF16)

            wq_sb = wts.tile([P, EC, HPC * DK], BF16)
            wk_sb = wts.tile([P, EC, HPC * DK], BF16)
            wv_sb = wts.tile([P, EC, HPC * DK], BF16)
            wp_sb = wts.tile([P, EC, E], BF16)
            xp = xin.tile([P, EC, T], BF16)

            qT = qkv_pool.tile([P, NPAIR, T], BF16)
            kT = qkv_pool.tile([P, NPAIR, T], BF16)
            vsb = qkv_pool.tile([P, TC, HPC * (DK + 1)], BF16)
            # ones column per head (row 64 of each P@V psum = softmax denominator)
            nc.vector.memset(
                vsb.rearrange("p t (h c) -> p t h c", c=DK + 1)[:, :, :, DK : DK + 1], 1.0
            )

            att2 = []
            for h in range(HPC):
                a = att_pool.tile([P, T], BF16, name=f"att2_{h}", tag=f"att2_{h}")
                att2.append(a)
                # last col of shifted half never written; keep sim happy
                nc.vector.memset(a[DK : 2 * DK, T - 1 : T], 0.0)

            # ---------------- input DMA (priority-chunked) ----------------
            dq = [nc.sync, nc.gpsimd, nc.scalar]
            _di = [0]

            def dma_in(out, in_):
                dq[_di[0] % 3].dma_start(out=out, in_=in_)
                _di[0] += 1

            # triplets (wk_e, wq_e, x_e[0:512]) so the first kT matmul can
            # start after ~256KB of traffic
            for e in range(EC):
                dma_in(wk_sb[:, e, :], wkT[ts(e, P), :])
                dma_in(wq_sb[:, e, :], wqT[ts(e, P), :])
                dma_in(xp[:, e, 0:TW], xT[ts(e, P), 0:TW])
            dma_in(bp_sb, bp[:, :])
            for e in range(EC):
                dma_in(wv_sb[:, e, :], wvT[ts(e, P), :])
            for e in range(EC):
                dma_in(xp[:, e, TW : 2 * TW], xT[ts(e, P), TW : 2 * TW])
            for e in range(EC):
                dma_in(xp[:, e, 2 * TW : 4 * TW], xT[ts(e, P), 2 * TW : 4 * TW])
            for e in range(EC):
                dma_in(wp_sb[:, e, :], wpT[ts(e, P), :])

            # ---------------- filler queue ----------------
            fillers = deque()
            n_added = [0]
            n_popped = [0]
            need = {}

            def add_f(fn):
                fillers.append(fn)
                n_added[0] += 1

            def take(n):
                while n > 0 and fillers:
                    fillers.popleft()()
                    n_popped[0] += 1
                    n -= 1

            def drain_to(k):
                while n_popped[0] < k and fillers:
                    fillers.popleft()()
                    n_popped[0] += 1

            def add_kq_group(dst, wsb, p, w):
                cell = {}

                for e in range(EC):
                    def mm(e=e, cell=cell, dst=dst, wsb=wsb, p=p, w=w):
                        if e == 0:
                            cell["ps"] = psq.tile([P, TW], F32, tag="qa", name="ps_qk")
                        nc.tensor.matmul(
                            cell["ps"],
                            wsb[:, e, ts(p, P)],
                            xp[:, e, ds(TW * w, TW)],
                            start=(e == 0),
                            stop=(e == EC - 1),
                        )

                    add_f(mm)

                def cp(cell=cell, dst=dst, p=p, w=w):
                    nc.vector.tensor_copy(dst[:, p, ds(TW * w, TW)], cell["ps"])

                add_f(cp)

            def add_v_group(t):
                cell = {}

                for e in range(EC):
                    def mm(e=e, cell=cell, t=t):
                        if e == 0:
                            cell["ps"] = psq.tile([P, HPC * DK], F32, tag="qa", name="ps_v")
                        nc.tensor.matmul(
                            cell["ps"],
                            xp[:, e, ts(t, P)],
                            wv_sb[:, e, :],
                            start=(e == 0),
                            stop=(e == EC - 1),
                        )

                    add_f(mm)

                def cp(cell=cell, t=t):
                    nc.vector.tensor_copy(
                        vsb.rearrange("p t (h c) -> p t h c", c=DK + 1)[:, t, :, 0:DK],
                        cell["ps"].rearrange("p (h d) -> p h d", d=DK),
                    )

                add_f(cp)

            def add_proj(h):
                a2v = att2[h].rearrange("p (r t) -> p r t", t=TT)
                for fw in range(FW):
                    cell = {}
                    for m in range(EC):
                        def mm(m=m, fw=fw, cell=cell, a2v=a2v):
                            if m == 0:
                                cell["yp"] = psq.tile([P, TW], F32, tag="qa", name="yp")
                            nc.tensor.matmul(
                                cell["yp"][0:RR, :],
                                a2v[:, :, 2 * m : 2 * m + 1],
                                wp_sb[:, m, ds(512 * fw, 512)],
                                start=(m == 0),
                                stop=False,
                            )

                        add_f(mm)

                    def mmb(fw=fw, cell=cell):
                        nc.tensor.matmul(
                            cell["yp"][0:RR, :],
                            ones[0:1, 0:RR],
                            bp_sb[0:1, ds(512 * fw, 512)],
                            start=False,
                            stop=True,
                        )

                    add_f(mmb)

                    def cpd(fw=fw, cell=cell, h=h):
                        ysb = ypool.tile([P, 512], BF16, name="ysb")
                        nc.vector.tensor_copy(ysb[0:RR, :], cell["yp"][0:RR, :])
                        nc.sync.dma_start(
                            out=y[ds(RR * h, RR), ds(512 * fw, 512)], in_=ysb[0:RR, :]
                        )

                    add_f(cpd)

            def add_kq(p, w):
                add_kq_group(kT, wk_sb, p, w)
                add_kq_group(qT, wq_sb, p, w)

            add_kq(0, 0)
            for t in range(0, 4):
                add_v_group(t)
            need[(0, 0)] = n_added[0]
            add_kq(0, 1)
            add_kq(1, 0)
            need[(1, 0)] = n_added[0]
            for t in range(4, 8):
                add_v_group(t)
            need[(0, 1)] = n_added[0]
            add_kq(0, 2)
            add_kq(1, 1)
            for t in range(8, 12):
                add_v_group(t)
            need[(0, 2)] = n_added[0]
            need[(1, 1)] = n_added[0]
            add_kq(0, 3)
            add_kq(1, 2)
            for t in range(12, 16):
                add_v_group(t)
            need[(0, 3)] = n_added[0]
            need[(1, 2)] = n_added[0]
            add_kq(1, 3)
            need[(1, 3)] = n_added[0]

            # ---------------- attention (pair-outer, pipelined) ----------------
            pending_norm = [None, None]

            for p in range(NPAIR):
                for w in range(W):
                    drain_to(need[(p, w)])
                    njc = JPW * (w + 1)
                    pvt = [
                        psa.tile([P, TW], F32, tag=f"pv{hl}", bufs=1, name=f"pv{hl}")
                        for hl in range(2)
                    ]
                    ess = {}

                    def emit_scores(jc, p=p, w=w, ess=ess):
                        qq = jc - JPW * w
                        trim = qq in (1, 2)
                        i0 = P * qq if trim else 0
                        st = psa.tile([P, 2 * TW], F32, tag="s", bufs=2, name="st")
                        for sub in range(2):
                            nc.tensor.matmul(
                                st[:, ds(TW * sub + i0, TW - i0)],
                                kT[ds(DK * sub, DK), p, ts(jc, P)],
                                qT[ds(DK * sub, DK), p, ds(TW * w + i0, TW - i0)],
                                start=True,
                                stop=True,
                            )
                        es = epool.tile([P, 2 * TW], BF16, name="es")
                        if trim:
                            for sub in range(2):
                                sl = ds(TW * sub + i0, TW - i0)
                                nc.scalar.activation(es[:, sl], st[:, sl], AF.Exp, scale=1.0 / 8.0)
                        else:
                            nc.scalar.activation(es, st, AF.Exp, scale=1.0 / 8.0)
                        if qq >= 0:
                            for sub in range(2):
                                sl = ds(TW * sub + i0, TW - i0)
                                nc.gpsimd.affine_select(
                                    out=es[:, sl],
                                    in_=es[:, sl],
                                    pattern=[[1, TW - i0]],
                                    compare_op=mybir.AluOpType.is_ge,
                                    fill=0.0,
                                    base=-(P * qq - i0),
                                    channel_multiplier=-1,
                                )
                        ess[jc] = (es, qq)

                    def emit_pv(jc, p=p, w=w, njc=njc, pvt=pvt, ess=ess):
                        es, qq = ess.pop(jc)
                        trim = qq in (1, 2)
                        i0 = P * qq if trim else 0
                        for hl in range(2):
                            h4 = 2 * p + hl
                            nc.tensor.matmul(
                                pvt[hl][0 : DK + 1, ds(i0, TW - i0)],
                                vsb[:, jc, ds((DK + 1) * h4, DK + 1)],
                                es[:, ds(TW * hl + i0, TW - i0)],
                                start=(jc == 0 and not trim),
                                stop=(jc == njc - 1 and not trim),
                                skip_group_check=trim,
                            )

                    for step in range(njc + 2):
                        if step < njc:
                            emit_scores(step)
                        take(2)
                        if step >= 2:
                            emit_pv(step - 2)
                        take(1)
                        if step == njc + 1 and pending_norm[1] is not None:
                            pending_norm[1]()  # rt broadcast + normalize + shift
                            pending_norm[1] = None

                    # ---- window drain: praw + denominators ----
                    dns = rpool.tile([P, 2 * TW], F32, name="dns", tag="dns")
                    praws = []
                    for hl in range(2):
                        praw = rpool.tile([P, TW], BF16, name="praw", tag=f"praw{hl}", bufs=2)
                        nc.vector.tensor_copy(praw[0:DK, :], pvt[hl][0:DK, :])
                        nc.vector.tensor_copy(
                            dns[DK : DK + 1, ds(TW * hl, TW)], pvt[hl][DK : DK + 1, :]
                        )
                        praws.append(praw)

                    recb = rpool.tile([P, 2 * TW], BF16, name="recb", tag="recb", bufs=2)

                    def norm_chain(p=p, w=w, dns=dns, recb=recb):
                        sp = rpool.tile([P, 2 * NSP], F32, name="sp", tag="sp")
                        nc.sync.dma_start(
                            out=sp[:, 0:NSP],
                            in_=dns[DK : DK + 1, :].rearrange("a (p c) -> a p c", c=NSP),
                        )
                        nc.vector.reciprocal(out=sp[:, NSP : 2 * NSP], in_=sp[:, 0:NSP])
                        spb = rpool.tile([P, 2 * NSP], BF16, name="spb", tag="spb")
                        nc.vector.tensor_copy(spb[:, 0:NSP], sp[:, NSP : 2 * NSP])
                        nc.sync.dma_start(
                            out=recb[DK : DK + 1, :].rearrange("a (p c) -> a p c", c=NSP),
                            in_=spb[:, 0:NSP],
                        )

                    def norm_pe(p=p, w=w, recb=recb, praws=praws):
                        for hl in range(2):
                            h = 2 * p + hl
                            rt = psa.tile([P, 2 * TW], F32, tag="s", bufs=2, name="rt")
                            nc.tensor.matmul(
                                rt[0:DK, 0:TW],
                                ones[DK : DK + 1, 0:DK],
                                recb[DK : DK + 1, ds(TW * hl, TW)],
                                start=True,
                                stop=True,
                            )
                            nc.vector.tensor_mul(
                                att2[h][0:DK, ds(TW * w, TW)],
                                rt[0:DK, 0:TW],
                                praws[hl][0:DK, :],
                            )
                            if w == 0:
                                nc.sync.dma_start(
                                    out=att2[h][DK : 2 * DK, 0 : TW - 1],
                                    in_=att2[h][0:DK, 1:TW],
                                )
                            else:
                                nc.sync.dma_start(
                                    out=att2[h][DK : 2 * DK, TW * w - 1 : TW * (w + 1) - 1],
                                    in_=att2[h][0:DK, ds(TW * w, TW)],
                                )
                            if w == W - 1:
                                # head complete: its projection becomes filler
                                add_proj(h)

                    norm_chain()  # reciprocal chain starts right away (DVE/DMA only)
                    pending_norm[1] = norm_pe

            # ---------------- tail ----------------
            # keep PE warm through the last norm chain, then flush projections
            take(10**9)

            def warm(n):
                wtf = psa.tile([P, 2 * TW], F32, tag="s", bufs=2, name="wtf")
                for i in range(n):
                    nc.tensor.matmul(
                        wtf[0 : DK + 1, 0:TW],
                        zer[:, 0 : DK + 1],
                        wsrc,
                        start=(i == 0),
                        stop=(i == n - 1),
                    )

            warm(20)
            pending_norm[1]()  # last window's rt + normalize, appends projections
            pending_norm[1] = None
            warm(6)
            take(10**9)
    nc.compile()
    return nc


_CACHE = {}
LAST_RESULT = None


def _get_nc(T=2048):
    key = ("nc", T)
    if key not in _CACHE:
        _CACHE[key] = build_nc(T=T)
    return _CACHE[key]


def make_in_maps(x, w_qkv, w_proj, b_proj):
    B, T, _E = x.shape
    in_maps = []
    wpTh = np.ascontiguousarray(w_proj.T.astype(BF16NP))
    bph = np.ascontiguousarray(b_proj.reshape(1, E).astype(BF16NP))
    xTs = [np.ascontiguousarray(x[b].T.astype(BF16NP)) for b in range(B)]
    for c in range(8):
        b, g = divmod(c, 4)
        r0 = HPC * DK * g  # 256*g
        sl = slice(r0, r0 + HPC * DK)
        in_maps.append(
            {
                "xT": xTs[b],
                "wqT": np.ascontiguousarray(w_qkv[sl, :].T.astype(BF16NP)),
                "wkT": np.ascontiguousarray(w_qkv[E:][sl, :].T.astype(BF16NP)),
                "wvT": np.ascontiguousarray(w_qkv[2 * E :][sl, :].T.astype(BF16NP)),
                "wpT": wpTh,
                "bp": bph,
            }
        )
    return in_maps


def kernel(x, w_qkv, w_proj, b_proj):
    global LAST_RESULT
    from concourse.bass_utils import run_bass_kernel_spmd

    x = np.asarray(x, dtype=np.float32)
    w_qkv = np.asarray(w_qkv, dtype=np.float32)
    w_proj = np.asarray(w_proj, dtype=np.float32)
    b_proj = np.asarray(b_proj, dtype=np.float32)
    B, T, _E = x.shape

    nc = _get_nc(T=T)
    in_maps = make_in_maps(x, w_qkv, w_proj, b_proj)
    res = run_bass_kernel_spmd(nc, in_maps, core_ids=list(range(8)))
    LAST_RESULT = res

    out = np.empty((B, T, E), dtype=np.float32)
    rows = HPC * ((T * DK) // E)  # 512 rows per core
    for c in range(8):
        b, g = divmod(c, 4)
        out[b, rows * g : rows * (g + 1), :] = np.asarray(res.results[c]["y"]).astype(np.float32)
    return out
